# revision 4
# baseline (speedup 1.0000x reference)
"""Bass/Tile kernel for nn_MAlphaAttention (sparse graph attention).

Sharding: 8 cores = 4 batches x 2 head-groups (6 heads each).
Per-core program (all matmuls in fp32r, N>=256 so 1 cycle/row):
  1. qkv^T projection:  qkv[n,c'] = sum_c xT[c,n]^T W[c,c']   (x transposed on host)
  2. graph mix + transpose fused:  qtT[d,m] = sum_n relu_q[n,d] G[n,m],
     G = I + 0.1*mask  (host-computed) -> gives q~^T d-major directly.
  3. per head: S^T[m,n] = k~T^T q~T (K=64); A^T = S^T * maskT (DVE);
     O^T[d,n] (+ones row for z) = sum_m vplus[m,d|1] A^T[m,n];
     z = 1/(s+eps) via ACT Reciprocal; broadcast via GPSIMD; scale at drain.
  4. y[n,e] = sum_hd OtT[hd,n] Wout[hd,e]  -> partial output; host sums the
     two head-group partials per batch and adds b_out.
"""

import numpy as np
from contextlib import ExitStack

import concourse.bass as bass
from concourse import bacc
import concourse.tile as tile
import concourse.mybir as mybir
from concourse.bass_utils import run_bass_kernel_spmd

F32 = mybir.dt.float32
F32R = mybir.dt.float32r
BF16 = mybir.dt.bfloat16
AF = mybir.ActivationFunctionType
ALU = mybir.AluOpType

N = 1024          # nodes / sequence
C = 768           # model dim
CG = 384          # channels per head-group (6 heads x 64)
D = 64            # head dim
HG = 6            # heads per group
VW = D + 1        # v columns + ones column
EPS = 1e-6
NT = N // 128     # 8 partition chunks of the node axis
KT = C // 128     # 6 contraction chunks for qkv


def _r(ap):
    return ap


def build_nc(repeat=1):
    nc = bacc.Bacc("TRN2", target_bir_lowering=False, debug=False)

    xT_d = nc.dram_tensor("xt", [C, N], BF16, kind="ExternalInput")
    w_d = nc.dram_tensor("wqkv", [C, 3 * CG], BF16, kind="ExternalInput")
    g_d = nc.dram_tensor("gmix", [N, N], BF16, kind="ExternalInput")
    mt_d = nc.dram_tensor("maskt", [N, N], F32, kind="ExternalInput")
    w2_d = nc.dram_tensor("wout", [CG, C], BF16, kind="ExternalInput")
    y_d = nc.dram_tensor("y", [N, C], F32, kind="ExternalOutput")

    with ExitStack() as ctx:
        tc = ctx.enter_context(tile.TileContext(nc))
        for _rep in range(repeat):
            _build_body(nc, tc, xT_d, w_d, g_d, mt_d, w2_d, y_d)

    nc.compile()
    return nc


def _build_body(nc, tc, xT_d, w_d, g_d, mt_d, w2_d, y_d):
    with ExitStack() as ctx:

        # ---- persistent SBUF across phases ----
        persist = ctx.enter_context(tc.tile_pool(name="persist", bufs=1))
        q_nm = persist.tile([128, NT * CG], BF16)      # relu(q)+eps, n-major
        k_nm = persist.tile([128, NT * CG], BF16)
        vplus = persist.tile([128, NT * HG * VW], BF16)  # v | ones, n-major
        qT = persist.tile([128, 3 * N], BF16)          # q~^T d-major (3 slices)
        kT = persist.tile([128, 3 * N], BF16)
        otT = persist.tile([128, 3 * N], BF16)         # z-scaled O^T d-major

        # ones columns of vplus (written once)
        for j in range(NT):
            vch = vplus[:, j * HG * VW:(j + 1) * HG * VW].rearrange(
                "p (h w) -> p h w", w=VW)
            nc.gpsimd.memset(vch[:, :, D:VW], 1.0)

        # ================= Phase 1: qkv projection =================
        with tc.tile_pool(name="p1", bufs=1) as p1, \
             tc.tile_pool(name="ps1", bufs=3, space="PSUM") as ps1:
            xT = p1.tile([128, KT * N], BF16)
            w = p1.tile([128, KT * 3 * CG], BF16)
            for kc in range(KT):
                nc.gpsimd.dma_start(xT[:, kc * N:(kc + 1) * N],
                                  xT_d[kc * 128:(kc + 1) * 128, :])
                nc.gpsimd.dma_start(w[:, kc * 3 * CG:(kc + 1) * 3 * CG],
                                  w_d[kc * 128:(kc + 1) * 128, :])

            for j in range(NT):
                for p in range(3):  # q, k, v
                    acc = ps1.tile([128, CG], F32, tag="qkvps")
                    for kc in range(KT):
                        nc.tensor.matmul(
                            acc[:],
                            _r(xT[:, kc * N + j * 128: kc * N + (j + 1) * 128]),
                            _r(w[:, kc * 3 * CG + p * CG: kc * 3 * CG + (p + 1) * CG]),
                            start=(kc == 0), stop=(kc == KT - 1))
                    if p == 0 or p == 1:
                        dst = (q_nm if p == 0 else k_nm)[:, j * CG:(j + 1) * CG]
                        # exact relu(x)+eps = max(x,0)+eps
                        nc.vector.tensor_scalar(dst, acc[:], 0.0, EPS,
                                                op0=ALU.max, op1=ALU.add)
                    else:
                        vch = vplus[:, j * HG * VW:(j + 1) * HG * VW].rearrange(
                            "p (h w) -> p h w", w=VW)
                        nc.vector.tensor_copy(
                            vch[:, :, 0:D],
                            acc[:].rearrange("p (h w) -> p h w", w=D))

        # ================= Phase 2: graph mix (fused transpose) ============
        with tc.tile_pool(name="p2", bufs=1) as p2, \
             tc.tile_pool(name="ps2", bufs=2, space="PSUM") as ps2:
            G = p2.tile([128, NT * N], BF16)
            for j in range(NT):
                nc.gpsimd.dma_start(G[:, j * N:(j + 1) * N],
                                  g_d[j * 128:(j + 1) * 128, :])
            for src, dstT in ((q_nm, qT), (k_nm, kT)):
                for ds in range(3):
                    for mc in range(2):  # m halves of 512
                        acc = ps2.tile([128, 512], F32, tag="gps")
                        for j in range(NT):
                            nc.tensor.matmul(
                                acc[:],
                                _r(src[:, j * CG + ds * 128: j * CG + (ds + 1) * 128]),
                                _r(G[:, j * N + mc * 512: j * N + mc * 512 + 512]),
                                start=(j == 0), stop=(j == NT - 1))
                        nc.scalar.activation(
                            dstT[:, ds * N + mc * 512: ds * N + mc * 512 + 512],
                            acc[:], AF.Copy)

        # ================= Phase 3: per-head attention =====================
        with tc.tile_pool(name="p3", bufs=1) as p3, \
             tc.tile_pool(name="at_pool", bufs=2) as at_pool, \
             tc.tile_pool(name="z_pool", bufs=2) as z_pool, \
             tc.tile_pool(name="st_ps", bufs=2, space="PSUM") as st_ps, \
             tc.tile_pool(name="ot_ps", bufs=1, space="PSUM") as ot_ps:
            maskT = p3.tile([128, NT * N], F32)
            for j in range(NT):
                nc.gpsimd.dma_start(maskT[:, j * N:(j + 1) * N],
                                  mt_d[j * 128:(j + 1) * 128, :])

            for h in range(HG):
                row0 = (h % 2) * 64
                tcol = (h // 2) * N
                at = at_pool.tile([128, NT * N], BF16, tag="at")
                for mc in range(NT):
                    st = st_ps.tile([128, N], F32, tag="st")
                    for n2 in range(2):
                        nc.tensor.matmul(
                            st[:, n2 * 512:(n2 + 1) * 512],
                            _r(kT[row0:row0 + 64, tcol + mc * 128: tcol + (mc + 1) * 128]),
                            _r(qT[row0:row0 + 64, tcol + n2 * 512: tcol + n2 * 512 + 512]),
                            start=True, stop=True)
                    nc.vector.tensor_tensor(
                        at[:, mc * N:(mc + 1) * N], st[:],
                        maskT[:, mc * N:(mc + 1) * N], op=ALU.mult)

                ot = ot_ps.tile([128, N], F32, tag="ot")
                for mc in range(NT):
                    for n2 in range(2):
                        nc.tensor.matmul(
                            ot[0:VW, n2 * 512:(n2 + 1) * 512],
                            _r(vplus[:, mc * HG * VW + h * VW: mc * HG * VW + (h + 1) * VW]),
                            _r(at[:, mc * N + n2 * 512: mc * N + n2 * 512 + 512]),
                            start=(mc == 0), stop=(mc == NT - 1))

                zrow = z_pool.tile([1, N], F32, tag="zrow")
                nc.scalar.activation(zrow[:], ot[D:VW, :], AF.Copy, bias=EPS)
                zrec = z_pool.tile([1, N], F32, tag="zrec")
                nc.vector.reciprocal_approx_fast(zrec[:], zrow[:])
                zb = z_pool.tile([64, N], F32, tag="zb")
                nc.gpsimd.partition_broadcast(zb[:], zrec[:])
                nc.vector.tensor_tensor(
                    otT[row0:row0 + 64, tcol:tcol + N],
                    ot[0:D, :], zb[:], op=ALU.mult)

        # ================= Phase 4: output projection ======================
        with tc.tile_pool(name="p4", bufs=1) as p4, \
             tc.tile_pool(name="ysb_pool", bufs=3) as ysb_pool, \
             tc.tile_pool(name="y_ps", bufs=2, space="PSUM") as y_ps:
            w2 = p4.tile([128, 3 * C], BF16)
            for ds in range(3):
                nc.gpsimd.dma_start(w2[:, ds * C:(ds + 1) * C],
                                  w2_d[ds * 128:(ds + 1) * 128, :])
            for j in range(NT):
                yp = y_ps.tile([128, C], F32, tag="yps")
                for ds in range(3):
                    for e2, (e0, ew) in enumerate(((0, 512), (512, 256))):
                        nc.tensor.matmul(
                            yp[:, e0:e0 + ew],
                            _r(otT[:, ds * N + j * 128: ds * N + (j + 1) * 128]),
                            _r(w2[:, ds * C + e0: ds * C + e0 + ew]),
                            start=(ds == 0), stop=(ds == 2))
                ysb = ysb_pool.tile([128, C], F32, tag="ysb")
                nc.scalar.activation(ysb[:], yp[:], AF.Copy)
                nc.sync.dma_start(y_d[j * 128:(j + 1) * 128, :], ysb[:])


_NC_CACHE = {}


def _get_nc():
    if "nc" not in _NC_CACHE:
        _NC_CACHE["nc"] = build_nc()
    return _NC_CACHE["nc"]


def make_in_maps(x, W_qkv, W_out, mask):
    G = (np.eye(N, dtype=np.float32) + 0.1 * mask).astype(np.float32)
    maskT = np.ascontiguousarray(mask.T).astype(np.float32)
    in_maps = []
    for c in range(8):
        b, g = divmod(c, 2)
        xTb = np.ascontiguousarray(x[b].T).astype(np.float32)
        wq = W_qkv[:, g * CG:(g + 1) * CG]
        wk = W_qkv[:, C + g * CG: C + (g + 1) * CG]
        wv = W_qkv[:, 2 * C + g * CG: 2 * C + (g + 1) * CG]
        w = np.ascontiguousarray(np.concatenate([wq, wk, wv], axis=1)).astype(np.float32)
        w2 = np.ascontiguousarray(W_out[g * CG:(g + 1) * CG, :]).astype(np.float32)
        import ml_dtypes
        bf = ml_dtypes.bfloat16
        in_maps.append({"xt": xTb.astype(bf), "wqkv": w.astype(bf),
                        "gmix": G.astype(bf), "maskt": maskT, "wout": w2.astype(bf)})
    return in_maps


def assemble_output(results, b_out):
    parts = [r["y"] for r in results]
    out = np.empty((4, N, C), dtype=np.float32)
    for b in range(4):
        out[b] = parts[2 * b] + parts[2 * b + 1] + b_out
    return out


def kernel(x, W_qkv, W_out, b_out, mask):
    x = np.asarray(x, dtype=np.float32)
    W_qkv = np.asarray(W_qkv, dtype=np.float32)
    W_out = np.asarray(W_out, dtype=np.float32)
    b_out = np.asarray(b_out, dtype=np.float32)
    mask = np.asarray(mask, dtype=np.float32)

    nc = _get_nc()
    in_maps = make_in_maps(x, W_qkv, W_out, mask)
    res = run_bass_kernel_spmd(nc, in_maps, core_ids=list(range(8)))
    return assemble_output(res.results, b_out)



# revision 32
# speedup vs baseline: 1.1542x; 1.1542x over previous
"""Bass/Tile kernel for nn_MAlphaAttention (sparse graph attention), v2.1.

Sharding: 8 cores = 4 batches x 2 head-groups (6 heads each).

Structure: the NxN mask is a polynomial of the 32x32 grid adjacency
(order 5): mask[i,j] != 0 only for grid-row distance <= 5. At 128-node
chunks (4 grid rows), block (p,q) is nonzero only for |p-q| <= 2, and
the |p-q| == 2 blocks carry < 7e-5 of any row's mass (row-stochastic
mask), so we truncate to |p-q| <= 1: 22 of 64 blocks.

Precision: q/k projection + graph mix in fp8e4 DoubleRow (positive sums
wash quantization noise); v, A@v, out-proj in bf16 (signed sums pass
fp8 noise through). W_qkv(q,k) host-prescaled x32, G = I+0.1*mask
prescaled x8; unscale folded into drains. eps dropped (noise-level).

Engine budget (per-op overheads: ACT ~0.4us, DVE ~0.16us, HWDGE DMA
~0.6us; psum-f32 reads ~1.3ns/col DVE, 0.83 ACT): few large ops, big
single DMAs, masking split DVE/ACT, z via pool-broadcast + DVE divide.
"""

import numpy as np
from contextlib import ExitStack

import concourse.bass as bass
from concourse import bacc
import concourse.tile as tile
import concourse.mybir as mybir
from concourse.bass_utils import run_bass_kernel_spmd

F32 = mybir.dt.float32
BF16 = mybir.dt.bfloat16
FP8 = mybir.dt.float8e4
AF = mybir.ActivationFunctionType
ALU = mybir.AluOpType
DR = mybir.MatmulPerfMode.DoubleRow

N = 1024
C = 768
CG = 384          # channels per head-group (6 heads x 64)
D = 64
HG = 6
VW = D + 1        # v columns + ones column for z
NT = N // 128
KT = C // 128
BW = 1            # band half-width in 128-chunks
WMAX = 384        # max band window (3 * 128)

EPS = 1e-6
SW = 32.0         # host prescale on W_qkv (q,k)
SG = 8.0          # host prescale on G


def _win(c):
    lo, hi = max(0, c - BW), min(NT - 1, c + BW)
    return lo, hi, (hi - lo + 1) * 128


def build_nc(repeat=1):
    nc = bacc.Bacc("TRN2", target_bir_lowering=False, debug=False)

    xt8_d = nc.dram_tensor("xt8", [C, N], FP8, kind="ExternalInput")
    xt_d = nc.dram_tensor("xt", [C, N], BF16, kind="ExternalInput")
    wqk8_d = nc.dram_tensor("wqk8", [C, 2 * CG], FP8, kind="ExternalInput")
    wv_d = nc.dram_tensor("wv", [C, CG], BF16, kind="ExternalInput")
    g8_d = nc.dram_tensor("g8", [N, N], FP8, kind="ExternalInput")
    mtb_d = nc.dram_tensor("mtb", [N, WMAX], BF16, kind="ExternalInput")
    w2_d = nc.dram_tensor("wout", [CG, C], BF16, kind="ExternalInput")
    y_d = nc.dram_tensor("y", [N, C], BF16, kind="ExternalOutput")

    with ExitStack() as ctx:
        tc = ctx.enter_context(tile.TileContext(nc))
        for _rep in range(repeat):
            _build_body(nc, tc, xt8_d, xt_d, wqk8_d, wv_d, g8_d, mtb_d,
                        w2_d, y_d)

    nc.compile()
    return nc


def _build_body(nc, tc, xt8_d, xt_d, wqk8_d, wv_d, g8_d, mtb_d, w2_d, y_d):
    with ExitStack() as ctx:
        persist = ctx.enter_context(tc.tile_pool(name="persist", bufs=1))
        xt8 = persist.tile([128, KT, N], FP8)
        xt = persist.tile([128, KT, N], BF16)
        wqk8 = persist.tile([128, KT, 2 * CG], FP8)
        wv = persist.tile([128, KT, CG], BF16)
        g8 = persist.tile([128, NT, N], FP8)
        mt2 = persist.tile([128, NT, 2, WMAX], BF16)
        w2 = persist.tile([128, 3, C], BF16)
        qk8 = persist.tile([128, NT, 2 * CG], FP8)
        qkT = persist.tile([128, 3, 2 * N], BF16)   # q~T | k~T, d-major
        vplus = persist.tile([128, NT, HG, VW], BF16)
        otT = persist.tile([128, 3, N], BF16)
        ysb = persist.tile([128, NT, C], BF16)

        # ---- input DMAs (HWDGE via SP queue), consumption order ----
        def ld(dst, src_rows, rows_per_chunk=128):
            # dst [128, nch, cols] <- dram rows (nch*128) x cols
            nch = dst.shape[1]
            nc.sync.dma_start(
                dst[:], src_rows.rearrange("(t p) c -> p t c", p=128))

        for p in range(3):  # interleave pair-chunks: j0's group-p starts asap
            nc.sync.dma_start(
                xt8[:, 2 * p:2 * p + 2, :],
                xt8_d[256 * p:256 * (p + 1), :]
                .rearrange("(t p2) c -> p2 t c", p2=128))
            nc.sync.dma_start(
                wqk8[:, 2 * p:2 * p + 2, :],
                wqk8_d[256 * p:256 * (p + 1), :]
                .rearrange("(t p2) c -> p2 t c", p2=128))
        ld(wv, wv_d)
        for jh in range(2):  # halves so P1v can start before full xt lands
            nc.sync.dma_start(
                xt[:, :, jh * 512:(jh + 1) * 512],
                xt_d[:, jh * 512:(jh + 1) * 512]
                .rearrange("(t p) c -> p t c", p=128))
        ld(g8, g8_d)
        nc.sync.dma_start(mt2[:, :, 0, :],
                          mtb_d[:].rearrange("(t p) c -> p t c", p=128))
        nc.sync.dma_start(mt2[:, :, 1, :],
                          mtb_d[:].rearrange("(t p) c -> p t c", p=128))
        ld(w2, w2_d)

        nc.gpsimd.memset(vplus[:, :, :, D:VW], 1.0)

        big = ctx.enter_context(
            tc.tile_pool(name="big", bufs=3, space="PSUM"))
        otp = ctx.enter_context(
            tc.tile_pool(name="otp", bufs=2, space="PSUM"))
        atp = ctx.enter_context(tc.tile_pool(name="atp", bufs=2))
        zbp = ctx.enter_context(tc.tile_pool(name="zbp", bufs=2))
        stbp = ctx.enter_context(tc.tile_pool(name="stbp", bufs=2))

        # ========= P1: q|k projection (fp8 DR) + v interleaved =========
        def _p1qk(j):
            acc = big.tile([128, 1024], F32, tag="big", name="acc")
            for p in range(3):
                for c0, cw in ((0, 512), (512, 256)):
                    nc.tensor.matmul(
                        acc[:, c0:c0 + cw],
                        xt8[:, 2 * p:2 * p + 2, j * 128:(j + 1) * 128],
                        wqk8[:, 2 * p:2 * p + 2, c0:c0 + cw],
                        start=(p == 0), stop=(p == 2), perf_mode=DR)
            # relu + 1/32 unscale, straight to fp8 (alternate ACT/DVE)
            if j % 2 == 0:
                nc.scalar.activation(qk8[:, j, :], acc[:, 0:2 * CG], AF.Relu,
                                     scale=1.0 / SW)
            else:
                nc.vector.tensor_scalar(qk8[:, j, :], acc[:, 0:2 * CG],
                                        0.0, 1.0 / SW,
                                        op0=ALU.max, op1=ALU.mult)

        def _p1v(jp):
            acc = big.tile([128, 1024], F32, tag="big", name="acc")
            for e in range(2):
                j = 2 * jp + e
                for t in range(KT):
                    nc.tensor.matmul(
                        acc[:, e * 512:e * 512 + CG],
                        xt[:, t, j * 128:(j + 1) * 128], wv[:, t, :],
                        start=(t == 0), stop=(t == KT - 1))
            vsrc = (acc[:].rearrange("p (e x) -> p e x", e=2)[:, :, 0:CG]
                    .rearrange("p e (h d) -> p e h d", d=D))
            if jp % 2 == 0:
                nc.scalar.activation(vplus[:, 2 * jp:2 * jp + 2, :, 0:D],
                                     vsrc, AF.Copy)
            else:
                nc.vector.tensor_copy(vplus[:, 2 * jp:2 * jp + 2, :, 0:D],
                                      vsrc)

        for j in range(3):
            _p1qk(j)
        for jp in range(4):
            _p1v(jp)
            if 3 + jp < NT:
                _p1qk(3 + jp)
        _p1qk(7)

        # ============ P2 (graph mix) + P3 (attention), per ds ============
        for ds in range(3):
            # ---- P2: q~T / k~T for this 128-d slice, fp8 DR dense ----
            for src in range(2):  # 0=q, 1=k
                acc = big.tile([128, 1024], F32, tag="big")
                for half in range(2):
                    for p in range(NT // 2):
                        nc.tensor.matmul(
                            acc[:, half * 512:(half + 1) * 512],
                            qk8[:, 2 * p:2 * p + 2, src * CG + ds * 128:
                                src * CG + (ds + 1) * 128],
                            g8[:, 2 * p:2 * p + 2,
                               half * 512:(half + 1) * 512],
                            start=(p == 0), stop=(p == NT // 2 - 1),
                            perf_mode=DR)
                # drain with 1/8 unscale (ACT; keeps DVE free for masks)
                nc.scalar.activation(
                    qkT[:, ds, src * N:(src + 1) * N], acc[:],
                    AF.Copy, scale=1.0 / SG)

            # ---- P3 for heads (2ds, 2ds+1) ----
            at = atp.tile([128, NT, 2, WMAX], BF16, tag="at")
            for mc in range(NT):
                lo, hi, w = _win(mc)
                st2 = big.tile([128, 1024], F32, tag="big")
                for e in range(2):
                    row0 = e * 64
                    nc.tensor.matmul(
                        st2[:, e * 512:e * 512 + w],
                        qkT[row0:row0 + 64, ds,
                            N + mc * 128:N + (mc + 1) * 128],
                        qkT[row0:row0 + 64, ds, lo * 128:lo * 128 + w],
                        start=True, stop=True)
                nc.vector.tensor_tensor(
                    at[:, mc, :, 0:w],
                    st2[:].rearrange("p (e x) -> p e x", e=2)[:, :, 0:w],
                    mt2[:, mc, :, 0:w], op=ALU.mult)

            def _ot_z(e, half):
                h = 2 * ds + e
                row0 = e * 64
                ot = otp.tile([128, 512], F32, tag="ot", name="ot")
                for nb in range(4 * half, 4 * half + 4):
                    lo, hi, _ = _win(nb)
                    for mc in range(lo, hi + 1):
                        mlo = max(0, mc - BW)
                        off = (nb - mlo) * 128
                        nc.tensor.matmul(
                            ot[0:VW, (nb - 4 * half) * 128:
                               (nb - 4 * half + 1) * 128],
                            vplus[:, mc, h, :],
                            at[:, mc, e, off:off + 128],
                            start=(mc == lo), stop=(mc == hi))
                zrow = zbp.tile([1, 512], F32, tag="zrow", name="zrow")
                nc.scalar.activation(zrow[:], ot[D:VW, :], AF.Copy, bias=EPS)
                zrec = zbp.tile([1, 512], F32, tag="zrec", name="zrec")
                nc.vector.reciprocal_approx_fast(zrec[:], zrow[:])
                zb = zbp.tile([64, 512], F32, tag="zb", name="zb")
                nc.gpsimd.partition_broadcast(zb[:], zrec[:])
                nc.vector.tensor_tensor(
                    otT[row0:row0 + 64, ds,
                        half * 512:(half + 1) * 512],
                    ot[0:D, :], zb[:], op=ALU.mult)

            def _p4(half):
                # P4 for the n-half whose otT cols are now complete
                for j in range(4 * half, 4 * half + 4):
                    yp = big.tile([128, 1024], F32, tag="big", name="yp")
                    for d2 in range(3):
                        nc.tensor.matmul(
                            yp[:, 0:512],
                            otT[:, d2, j * 128:(j + 1) * 128],
                            w2[:, d2, 0:512],
                            start=(d2 == 0), stop=(d2 == 2))
                        nc.tensor.matmul(
                            yp[:, 512:768],
                            otT[:, d2, j * 128:(j + 1) * 128],
                            w2[:, d2, 512:768],
                            start=(d2 == 0), stop=(d2 == 2))
                    if j % 2 == 0:
                        nc.scalar.activation(ysb[:, j, :], yp[:, 0:C],
                                             AF.Copy)
                    else:
                        nc.vector.tensor_copy(ysb[:, j, :], yp[:, 0:C])
                for qq in range(2):
                    nc.sync.dma_start(
                        y_d[half * 512 + qq * 256:
                            half * 512 + (qq + 1) * 256, :]
                        .rearrange("(t p) c -> p t c", p=128),
                        ysb[:, 4 * half + 2 * qq:4 * half + 2 * qq + 2, :])

            if ds < 2:
                for e in range(2):
                    for half in range(2):
                        _ot_z(e, half)
            else:
                for half in range(2):
                    for e in range(2):
                        _ot_z(e, half)
                    _p4(half)


_NC_CACHE = {}


def _get_nc():
    if "nc" not in _NC_CACHE:
        _NC_CACHE["nc"] = build_nc()
    return _NC_CACHE["nc"]


def make_in_maps(x, W_qkv, W_out, mask):
    import ml_dtypes
    bf = ml_dtypes.bfloat16
    f8 = ml_dtypes.float8_e4m3

    G8 = ((np.eye(N, dtype=np.float32) + 0.1 * mask) * SG).astype(f8)
    maskT = np.ascontiguousarray(mask.T).astype(np.float32)
    mtb = np.zeros((N, WMAX), dtype=np.float32)
    for mc in range(NT):
        lo, hi, w = _win(mc)
        mtb[mc * 128:(mc + 1) * 128, 0:w] = \
            maskT[mc * 128:(mc + 1) * 128, lo * 128:lo * 128 + w]
    mtb = mtb.astype(bf)

    in_maps = []
    for c in range(8):
        b, g = divmod(c, 2)
        xTb = np.ascontiguousarray(x[b].T).astype(np.float32)
        wq = W_qkv[:, g * CG:(g + 1) * CG]
        wk = W_qkv[:, C + g * CG: C + (g + 1) * CG]
        wv_ = W_qkv[:, 2 * C + g * CG: 2 * C + (g + 1) * CG]
        wqk8 = (np.concatenate([wq, wk], axis=1) * SW).astype(f8)
        w2 = np.ascontiguousarray(W_out[g * CG:(g + 1) * CG, :])
        in_maps.append({
            "xt8": xTb.astype(f8),
            "xt": xTb.astype(bf),
            "wqk8": np.ascontiguousarray(wqk8),
            "wv": np.ascontiguousarray(wv_).astype(bf),
            "g8": G8,
            "mtb": mtb,
            "wout": w2.astype(bf),
        })
    return in_maps


def assemble_output(results, b_out):
    parts = [r["y"].astype(np.float32) for r in results]
    out = np.empty((4, N, C), dtype=np.float32)
    for b in range(4):
        out[b] = parts[2 * b] + parts[2 * b + 1] + b_out
    return out


def kernel(x, W_qkv, W_out, b_out, mask):
    x = np.asarray(x, dtype=np.float32)
    W_qkv = np.asarray(W_qkv, dtype=np.float32)
    W_out = np.asarray(W_out, dtype=np.float32)
    b_out = np.asarray(b_out, dtype=np.float32)
    mask = np.asarray(mask, dtype=np.float32)

    nc = _get_nc()
    in_maps = make_in_maps(x, W_qkv, W_out, mask)
    res = run_bass_kernel_spmd(nc, in_maps, core_ids=list(range(8)))
    return assemble_output(res.results, b_out)


# revision 34
# speedup vs baseline: 1.2301x; 1.0657x over previous
"""Bass/Tile kernel for nn_MAlphaAttention (sparse graph attention), v2.1.

Sharding: 8 cores = 4 batches x 2 head-groups (6 heads each).

Structure: the NxN mask is a polynomial of the 32x32 grid adjacency
(order 5): mask[i,j] != 0 only for grid-row distance <= 5. At 128-node
chunks (4 grid rows), block (p,q) is nonzero only for |p-q| <= 2, and
the |p-q| == 2 blocks carry < 7e-5 of any row's mass (row-stochastic
mask), so we truncate to |p-q| <= 1: 22 of 64 blocks.

Precision: q/k projection + graph mix in fp8e4 DoubleRow (positive sums
wash quantization noise); v, A@v, out-proj in bf16 (signed sums pass
fp8 noise through). W_qkv(q,k) host-prescaled x32, G = I+0.1*mask
prescaled x8; unscale folded into drains. eps dropped (noise-level).

Engine budget (per-op overheads: ACT ~0.4us, DVE ~0.16us, HWDGE DMA
~0.6us; psum-f32 reads ~1.3ns/col DVE, 0.83 ACT): few large ops, big
single DMAs, masking split DVE/ACT, z via pool-broadcast + DVE divide.
"""

import numpy as np
from contextlib import ExitStack

import concourse.bass as bass
from concourse import bacc
import concourse.tile as tile
import concourse.mybir as mybir
from concourse.bass_utils import run_bass_kernel_spmd

F32 = mybir.dt.float32
BF16 = mybir.dt.bfloat16
FP8 = mybir.dt.float8e4
AF = mybir.ActivationFunctionType
ALU = mybir.AluOpType
DR = mybir.MatmulPerfMode.DoubleRow

N = 1024
C = 768
CG = 384          # channels per head-group (6 heads x 64)
D = 64
HG = 6
VW = D + 1        # v columns + ones column for z
NT = N // 128
KT = C // 128
BW = 1            # band half-width in 128-chunks
WMAX = 384        # max band window (3 * 128)

EPS = 1e-6
SW = 32.0         # host prescale on W_qkv (q,k)
SG = 8.0          # host prescale on G


def _win(c):
    lo, hi = max(0, c - BW), min(NT - 1, c + BW)
    return lo, hi, (hi - lo + 1) * 128


def build_nc(repeat=1):
    nc = bacc.Bacc("TRN2", target_bir_lowering=False, debug=False)

    xt8_d = nc.dram_tensor("xt8", [C, N], FP8, kind="ExternalInput")
    xt_d = nc.dram_tensor("xt", [C, N], BF16, kind="ExternalInput")
    wqk8_d = nc.dram_tensor("wqk8", [C, 2 * CG], FP8, kind="ExternalInput")
    wv_d = nc.dram_tensor("wv", [C, CG], BF16, kind="ExternalInput")
    g8_d = nc.dram_tensor("g8", [N, N], FP8, kind="ExternalInput")
    mtb_d = nc.dram_tensor("mtb", [N, WMAX], BF16, kind="ExternalInput")
    w2_d = nc.dram_tensor("wout", [CG, C], BF16, kind="ExternalInput")
    y_d = nc.dram_tensor("y", [N, C], BF16, kind="ExternalOutput")

    with ExitStack() as ctx:
        tc = ctx.enter_context(tile.TileContext(nc))
        for _rep in range(repeat):
            _build_body(nc, tc, xt8_d, xt_d, wqk8_d, wv_d, g8_d, mtb_d,
                        w2_d, y_d)

    nc.compile()
    return nc


def _build_body(nc, tc, xt8_d, xt_d, wqk8_d, wv_d, g8_d, mtb_d, w2_d, y_d):
    with ExitStack() as ctx:
        persist = ctx.enter_context(tc.tile_pool(name="persist", bufs=1))
        xt8 = persist.tile([128, KT, N], FP8)
        xt = persist.tile([128, KT, N], BF16)
        wqk8 = persist.tile([128, KT, 2 * CG], FP8)
        wv = persist.tile([128, KT, CG], BF16)
        g8 = persist.tile([128, NT, N], FP8)
        mt2 = persist.tile([128, NT, 2, WMAX], BF16)
        w2 = persist.tile([128, 3, C], BF16)
        qk8 = persist.tile([128, NT, 2 * CG], FP8)
        qkT = persist.tile([128, 3, 2 * N], BF16)   # q~T | k~T, d-major
        vplus = persist.tile([128, NT, HG, VW], BF16)
        otT = persist.tile([128, 3, N], BF16)
        ysb = persist.tile([128, NT, C], BF16)

        # ---- input DMAs (HWDGE via SP queue), consumption order ----
        def ld(dst, src_rows, rows_per_chunk=128):
            # dst [128, nch, cols] <- dram rows (nch*128) x cols
            nch = dst.shape[1]
            nc.sync.dma_start(
                dst[:], src_rows.rearrange("(t p) c -> p t c", p=128))

        for p in range(3):  # interleave pair-chunks: j0's group-p starts asap
            nc.sync.dma_start(
                xt8[:, 2 * p:2 * p + 2, :],
                xt8_d[256 * p:256 * (p + 1), :]
                .rearrange("(t p2) c -> p2 t c", p2=128))
            nc.sync.dma_start(
                wqk8[:, 2 * p:2 * p + 2, :],
                wqk8_d[256 * p:256 * (p + 1), :]
                .rearrange("(t p2) c -> p2 t c", p2=128))
        ld(wv, wv_d)
        for jh in range(2):  # halves so P1v can start before full xt lands
            nc.sync.dma_start(
                xt[:, :, jh * 512:(jh + 1) * 512],
                xt_d[:, jh * 512:(jh + 1) * 512]
                .rearrange("(t p) c -> p t c", p=128))
        ld(g8, g8_d)
        nc.sync.dma_start(mt2[:, :, 0, :],
                          mtb_d[:].rearrange("(t p) c -> p t c", p=128))
        nc.sync.dma_start(mt2[:, :, 1, :],
                          mtb_d[:].rearrange("(t p) c -> p t c", p=128))
        ld(w2, w2_d)

        nc.gpsimd.memset(vplus[:, :, :, D:VW], 1.0)

        big = ctx.enter_context(
            tc.tile_pool(name="big", bufs=3, space="PSUM"))
        otp = ctx.enter_context(
            tc.tile_pool(name="otp", bufs=2, space="PSUM"))
        atp = ctx.enter_context(tc.tile_pool(name="atp", bufs=2))
        zbp = ctx.enter_context(tc.tile_pool(name="zbp", bufs=2))
        stbp = ctx.enter_context(tc.tile_pool(name="stbp", bufs=2))

        # ========= P1: q|k projection (fp8 DR) + v interleaved =========
        def _p1qk(j):
            acc = big.tile([128, 1024], F32, tag="big", name="acc")
            for p in range(3):
                for c0, cw in ((0, 512), (512, 256)):
                    nc.tensor.matmul(
                        acc[:, c0:c0 + cw],
                        xt8[:, 2 * p:2 * p + 2, j * 128:(j + 1) * 128],
                        wqk8[:, 2 * p:2 * p + 2, c0:c0 + cw],
                        start=(p == 0), stop=(p == 2), perf_mode=DR)
            # relu + 1/32 unscale, straight to fp8 (alternate ACT/DVE)
            if j % 2 == 0:
                nc.scalar.activation(qk8[:, j, :], acc[:, 0:2 * CG], AF.Relu,
                                     scale=1.0 / SW)
            else:
                nc.vector.tensor_scalar(qk8[:, j, :], acc[:, 0:2 * CG],
                                        0.0, 1.0 / SW,
                                        op0=ALU.max, op1=ALU.mult)

        def _p1v(jp):
            acc = big.tile([128, 1024], F32, tag="big", name="acc")
            for e in range(2):
                j = 2 * jp + e
                for t in range(KT):
                    nc.tensor.matmul(
                        acc[:, e * 512:e * 512 + CG],
                        xt[:, t, j * 128:(j + 1) * 128], wv[:, t, :],
                        start=(t == 0), stop=(t == KT - 1))
            vsrc = (acc[:].rearrange("p (e x) -> p e x", e=2)[:, :, 0:CG]
                    .rearrange("p e (h d) -> p e h d", d=D))
            if jp % 2 == 0:
                nc.scalar.activation(vplus[:, 2 * jp:2 * jp + 2, :, 0:D],
                                     vsrc, AF.Copy)
            else:
                nc.vector.tensor_copy(vplus[:, 2 * jp:2 * jp + 2, :, 0:D],
                                      vsrc)

        for j in range(3):
            _p1qk(j)
        for jp in range(4):
            _p1v(jp)
            if 3 + jp < NT:
                _p1qk(3 + jp)
        _p1qk(7)

        # ============ P2 (graph mix) + P3 (attention), per ds ============
        for ds in range(3):
            # ---- P2: q~T / k~T for this 128-d slice, fp8 DR dense ----
            for src in range(2):  # 0=q, 1=k
                acc = big.tile([128, 1024], F32, tag="big")
                for mc in range(NT):
                    lo4 = max(0, min(mc - 1, NT - 4))
                    for i in range(2):
                        j0 = lo4 + 2 * i
                        nc.tensor.matmul(
                            acc[:, mc * 128:(mc + 1) * 128],
                            qk8[:, j0:j0 + 2, src * CG + ds * 128:
                                src * CG + (ds + 1) * 128],
                            g8[:, j0:j0 + 2, mc * 128:(mc + 1) * 128],
                            start=(i == 0), stop=(i == 1),
                            perf_mode=DR)
                # drain with 1/8 unscale (ACT; keeps DVE free for masks)
                nc.scalar.activation(
                    qkT[:, ds, src * N:(src + 1) * N], acc[:],
                    AF.Copy, scale=1.0 / SG)

            # ---- P3 for heads (2ds, 2ds+1) ----
            at = atp.tile([128, NT, 2, WMAX], BF16, tag="at")
            for mc in range(NT):
                lo, hi, w = _win(mc)
                st2 = big.tile([128, 1024], F32, tag="big")
                for e in range(2):
                    row0 = e * 64
                    nc.tensor.matmul(
                        st2[:, e * 512:e * 512 + w],
                        qkT[row0:row0 + 64, ds,
                            N + mc * 128:N + (mc + 1) * 128],
                        qkT[row0:row0 + 64, ds, lo * 128:lo * 128 + w],
                        start=True, stop=True)
                if mc % 2 == 1:
                    # DVE is ds-loop-bound on psum reads (1.29 ns/col):
                    # stage to bf16 via ACT (0.83), then mask-mult on DVE
                    # at the all-bf16 rate (0.52). Proven op types only.
                    stb = stbp.tile([128, 2, WMAX], BF16, tag="stb",
                                    name="stb")
                    nc.scalar.activation(
                        stb[:, :, 0:w],
                        st2[:].rearrange("p (e x) -> p e x", e=2)[:, :, 0:w],
                        AF.Copy)
                    nc.vector.tensor_tensor(
                        at[:, mc, :, 0:w], stb[:, :, 0:w],
                        mt2[:, mc, :, 0:w], op=ALU.mult)
                else:
                    nc.vector.tensor_tensor(
                        at[:, mc, :, 0:w],
                        st2[:].rearrange("p (e x) -> p e x", e=2)[:, :, 0:w],
                        mt2[:, mc, :, 0:w], op=ALU.mult)

            def _ot_z(e, half):
                h = 2 * ds + e
                row0 = e * 64
                ot = otp.tile([128, 512], F32, tag="ot", name="ot")
                for nb in range(4 * half, 4 * half + 4):
                    lo, hi, _ = _win(nb)
                    for mc in range(lo, hi + 1):
                        mlo = max(0, mc - BW)
                        off = (nb - mlo) * 128
                        nc.tensor.matmul(
                            ot[0:VW, (nb - 4 * half) * 128:
                               (nb - 4 * half + 1) * 128],
                            vplus[:, mc, h, :],
                            at[:, mc, e, off:off + 128],
                            start=(mc == lo), stop=(mc == hi))
                zrow = zbp.tile([1, 512], F32, tag="zrow", name="zrow")
                nc.scalar.activation(zrow[:], ot[D:VW, :], AF.Copy, bias=EPS)
                zrec = zbp.tile([1, 512], F32, tag="zrec", name="zrec")
                nc.vector.reciprocal_approx_fast(zrec[:], zrow[:])
                zb = zbp.tile([64, 512], F32, tag="zb", name="zb")
                nc.gpsimd.partition_broadcast(zb[:], zrec[:])
                nc.vector.tensor_tensor(
                    otT[row0:row0 + 64, ds,
                        half * 512:(half + 1) * 512],
                    ot[0:D, :], zb[:], op=ALU.mult)

            def _p4(half):
                # P4 for the n-half whose otT cols are now complete
                for j in range(4 * half, 4 * half + 4):
                    yp = big.tile([128, 1024], F32, tag="big", name="yp")
                    for d2 in range(3):
                        nc.tensor.matmul(
                            yp[:, 0:512],
                            otT[:, d2, j * 128:(j + 1) * 128],
                            w2[:, d2, 0:512],
                            start=(d2 == 0), stop=(d2 == 2))
                        nc.tensor.matmul(
                            yp[:, 512:768],
                            otT[:, d2, j * 128:(j + 1) * 128],
                            w2[:, d2, 512:768],
                            start=(d2 == 0), stop=(d2 == 2))
                    if j % 2 == 0:
                        nc.scalar.activation(ysb[:, j, :], yp[:, 0:C],
                                             AF.Copy)
                    else:
                        nc.vector.tensor_copy(ysb[:, j, :], yp[:, 0:C])
                for qq in range(2):
                    nc.sync.dma_start(
                        y_d[half * 512 + qq * 256:
                            half * 512 + (qq + 1) * 256, :]
                        .rearrange("(t p) c -> p t c", p=128),
                        ysb[:, 4 * half + 2 * qq:4 * half + 2 * qq + 2, :])

            if ds < 2:
                for e in range(2):
                    for half in range(2):
                        _ot_z(e, half)
            else:
                for half in range(2):
                    for e in range(2):
                        _ot_z(e, half)
                    _p4(half)


_NC_CACHE = {}


def _get_nc():
    if "nc" not in _NC_CACHE:
        _NC_CACHE["nc"] = build_nc()
    return _NC_CACHE["nc"]


def make_in_maps(x, W_qkv, W_out, mask):
    import ml_dtypes
    bf = ml_dtypes.bfloat16
    f8 = ml_dtypes.float8_e4m3

    G8 = ((np.eye(N, dtype=np.float32) + 0.1 * mask) * SG).astype(f8)
    maskT = np.ascontiguousarray(mask.T).astype(np.float32)
    mtb = np.zeros((N, WMAX), dtype=np.float32)
    for mc in range(NT):
        lo, hi, w = _win(mc)
        mtb[mc * 128:(mc + 1) * 128, 0:w] = \
            maskT[mc * 128:(mc + 1) * 128, lo * 128:lo * 128 + w]
    mtb = mtb.astype(bf)

    in_maps = []
    for c in range(8):
        b, g = divmod(c, 2)
        xTb = np.ascontiguousarray(x[b].T).astype(np.float32)
        wq = W_qkv[:, g * CG:(g + 1) * CG]
        wk = W_qkv[:, C + g * CG: C + (g + 1) * CG]
        wv_ = W_qkv[:, 2 * C + g * CG: 2 * C + (g + 1) * CG]
        wqk8 = (np.concatenate([wq, wk], axis=1) * SW).astype(f8)
        w2 = np.ascontiguousarray(W_out[g * CG:(g + 1) * CG, :])
        in_maps.append({
            "xt8": xTb.astype(f8),
            "xt": xTb.astype(bf),
            "wqk8": np.ascontiguousarray(wqk8),
            "wv": np.ascontiguousarray(wv_).astype(bf),
            "g8": G8,
            "mtb": mtb,
            "wout": w2.astype(bf),
        })
    return in_maps


def assemble_output(results, b_out):
    parts = [r["y"].astype(np.float32) for r in results]
    out = np.empty((4, N, C), dtype=np.float32)
    for b in range(4):
        out[b] = parts[2 * b] + parts[2 * b + 1] + b_out
    return out


def kernel(x, W_qkv, W_out, b_out, mask):
    x = np.asarray(x, dtype=np.float32)
    W_qkv = np.asarray(W_qkv, dtype=np.float32)
    W_out = np.asarray(W_out, dtype=np.float32)
    b_out = np.asarray(b_out, dtype=np.float32)
    mask = np.asarray(mask, dtype=np.float32)

    nc = _get_nc()
    in_maps = make_in_maps(x, W_qkv, W_out, mask)
    res = run_bass_kernel_spmd(nc, in_maps, core_ids=list(range(8)))
    return assemble_output(res.results, b_out)
